# revision 3
# baseline (speedup 1.0000x reference)
import numpy as np
import concourse.bass as bass
import concourse.mybir as mybir
import concourse.tile as tile
from concourse import bass_utils
import bass_rust

B, E, M, V, NSTEP = 64, 512, 64, 32000, 64
NC = 8
PROJ_ROWS = 4 * M * E          # 131072 rows of concatenated proj weights
PR_SHARD = PROJ_ROWS // NC     # 16384 rows/core
V_SHARD = V // NC              # 4000 vocab rows/core
NT = 500                       # vocab n-tile (fits one PSUM bank)
ROWS = B * NSTEP               # 4096 zs rows


def _split_multi_waits(nc, max_waits=1):
    # walrus in this container rejects >1 sem-wait on CTRL_NO instructions;
    # move extra waits onto preceding NoOps on the same engine.
    for f in nc.m.functions:
        for bb in f.blocks:
            new_insts = []
            for inst in bb.instructions:
                si = inst.sync_info
                if si is not None and si.on_wait and len(si.on_wait) > max_waits:
                    waits = list(si.on_wait)
                    head, tail = waits[:-max_waits], waits[-max_waits:]
                    for i in range(0, len(head), max_waits):
                        new_insts.append(mybir.InstNoOp(
                            name=f"{inst.name}_wsplit_{i}",
                            engine=inst.engine,
                            sync_info=bass_rust.SyncInfo(
                                on_wait=head[i:i + max_waits], on_update=[]),
                        ))
                    inst.sync_info = bass_rust.SyncInfo(
                        on_wait=tail, on_update=list(si.on_update))
                new_insts.append(inst)
            if len(new_insts) != len(bb.instructions):
                bb.instructions[:] = new_insts


def _build_proj_kernel():
    nc = bass.Bass("TRN2", target_bir_lowering=False, debug=False)
    z = nc.dram_tensor("z0t", [E, B], mybir.dt.float32, kind="ExternalInput")
    w = nc.dram_tensor("wt", [E, PR_SHARD], mybir.dt.float32, kind="ExternalInput")
    o = nc.dram_tensor("out", [B, PR_SHARD], mybir.dt.float32, kind="ExternalOutput")
    with tile.TileContext(nc) as tc:
        with tc.tile_pool(name="zp", bufs=1) as zp, \
             tc.tile_pool(name="wp", bufs=3) as wp, \
             tc.tile_pool(name="op", bufs=3) as op, \
             tc.tile_pool(name="ps", bufs=2, space="PSUM") as pp:
            zt = zp.tile([128, 4 * B], mybir.dt.float32)
            for c in range(4):
                nc.sync.dma_start(zt[:, c * B:(c + 1) * B],
                                  z[c * 128:(c + 1) * 128, :])
            for n in range(PR_SHARD // 512):
                wt = wp.tile([128, 4 * 512], mybir.dt.float32, tag="w")
                for c in range(4):
                    nc.sync.dma_start(
                        wt[:, c * 512:(c + 1) * 512],
                        w[c * 128:(c + 1) * 128, n * 512:(n + 1) * 512])
                ps = pp.tile([B, 512], mybir.dt.float32)
                for c in range(4):
                    nc.tensor.matmul(ps[:], zt[:, c * B:(c + 1) * B],
                                     wt[:, c * 512:(c + 1) * 512],
                                     start=(c == 0), stop=(c == 3))
                ot = op.tile([B, 512], mybir.dt.float32, tag="o")
                nc.vector.tensor_copy(ot[:], ps[:])
                nc.sync.dma_start(o[:, n * 512:(n + 1) * 512], ot[:])
    _split_multi_waits(nc)
    return nc


def _build_vocab_kernel():
    nc = bass.Bass("TRN2", target_bir_lowering=False, debug=False)
    z = nc.dram_tensor("zst", [E, ROWS], mybir.dt.float32, kind="ExternalInput")
    w = nc.dram_tensor("wvt", [E, V_SHARD], mybir.dt.float32, kind="ExternalInput")
    o = nc.dram_tensor("sig", [ROWS, V_SHARD // NT], mybir.dt.float32,
                       kind="ExternalOutput")
    NTILES = V_SHARD // NT
    with tile.TileContext(nc) as tc:
        with tc.tile_pool(name="stage", bufs=2) as sp, \
             tc.tile_pool(name="zr", bufs=1) as zrp, \
             tc.tile_pool(name="wr", bufs=1) as wrp, \
             tc.tile_pool(name="sg", bufs=3) as sgp, \
             tc.tile_pool(name="ex", bufs=2) as exp_, \
             tc.tile_pool(name="ps", bufs=4, space="PSUM") as pp:
        # round both operands to f32r once, keep resident in SBUF
            zr = [zrp.tile([128, ROWS], mybir.dt.float32r, name=f"zr{c}", tag=f"zr{c}")
                  for c in range(4)]
            wr = [wrp.tile([128, V_SHARD], mybir.dt.float32r, name=f"wr{c}", tag=f"wr{c}")
                  for c in range(4)]
            for c in range(4):
                st = sp.tile([128, ROWS], mybir.dt.float32, tag="st")
                nc.sync.dma_start(st[:], z[c * 128:(c + 1) * 128, :])
                nc.vector.tensor_copy(zr[c][:], st[:])
            for c in range(4):
                st = sp.tile([128, V_SHARD], mybir.dt.float32, tag="st")
                nc.sync.dma_start(st[:], w[c * 128:(c + 1) * 128, :])
                nc.vector.tensor_copy(wr[c][:], st[:])
            for m in range(ROWS // 128):
                sg = sgp.tile([128, NTILES], mybir.dt.float32, tag="sg")
                for n in range(NTILES):
                    ps = pp.tile([128, NT], mybir.dt.float32)
                    for c in range(4):
                        nc.tensor.matmul(
                            ps[:], zr[c][:, m * 128:(m + 1) * 128],
                            wr[c][:, n * NT:(n + 1) * NT],
                            start=(c == 0), stop=(c == 3))
                    et = exp_.tile([128, NT], mybir.dt.float32, tag="et")
                    nc.scalar.activation(et[:], ps[:],
                                         mybir.ActivationFunctionType.Exp,
                                         accum_out=sg[:, n:n + 1])
                nc.sync.dma_start(o[m * 128:(m + 1) * 128, :], sg[:])
    _split_multi_waits(nc)
    return nc


_CACHE = {}
_LAST_MAPS = {}


def _run(key, builder, in_maps):
    if key not in _CACHE:
        _CACHE[key] = builder()
    _LAST_MAPS[key] = in_maps
    return bass_utils.run_bass_kernel_spmd(
        _CACHE[key], in_maps, core_ids=list(range(NC)))


def _std_norm(x):
    s = x.std(axis=-1, keepdims=True, ddof=1)
    return x / (1e-5 + s) * 0.113


def kernel(zi, y, noise, latent, emit_k_w, emit_k_b, emit_v_w, emit_v_b,
           trans_k_w, trans_k_b, trans_v_w, trans_v_b, vocab_w, vocab_b):
    zi = np.asarray(zi); y = np.asarray(y)
    noise = np.asarray(noise, np.float32)
    latent = np.asarray(latent, np.float32)

    lat = latent[zi].reshape(B, 2, E)
    lat = _std_norm(lat) + (noise - 0.5) * np.float32(0.05)
    z0 = lat[:, 0]
    z = lat[:, 1:2].copy()

    # ---- phase 1: 4 projections on device, row-sharded over 8 cores ----
    wcat = np.concatenate([np.asarray(w_, np.float32) for w_ in
                           (emit_k_w, emit_v_w, trans_k_w, trans_v_w)], axis=0)
    z0t = np.ascontiguousarray(z0.T)
    in_maps = [{"z0t": z0t,
                "wt": np.ascontiguousarray(wcat[c * PR_SHARD:(c + 1) * PR_SHARD].T)}
               for c in range(NC)]
    res = _run("proj", _build_proj_kernel, in_maps)
    pcat = np.concatenate([res.results[c]["out"] for c in range(NC)], axis=1)
    ek, ev, tk, tv = [pcat[:, i * M * E:(i + 1) * M * E].reshape(B, M, E)
                      for i in range(4)]
    ek = ek + np.asarray(emit_k_b, np.float32).reshape(1, M, E)
    ev = ev + np.asarray(emit_v_b, np.float32).reshape(1, M, E)
    tk = tk + np.asarray(trans_k_b, np.float32).reshape(1, M, E)
    tv = tv + np.asarray(trans_v_b, np.float32).reshape(1, M, E)

    # ---- phase 2: the 64-step recurrence (tiny: 64x(64,512) state) ----
    ekT = ek.transpose(0, 2, 1); tkT = tk.transpose(0, 2, 1)
    zs = np.empty((B, NSTEP, E), np.float32)
    for t in range(NSTEP):
        zn = _std_norm(z)
        le = np.matmul(zn, ekT)
        le -= le.max(axis=-1, keepdims=True)
        ae = np.exp(le); ae /= ae.sum(axis=-1, keepdims=True)
        zs[:, t] = np.matmul(ae, ev)[:, 0]
        lt = np.matmul(zn, tkT)
        lt -= lt.max(axis=-1, keepdims=True)
        at = np.exp(lt); at /= at.sum(axis=-1, keepdims=True)
        z = np.matmul(at, tv)

    # ---- phase 3: vocab head on device, vocab-sharded over 8 cores ----
    vb = np.asarray(vocab_b, np.float32)
    zsf = zs.reshape(ROWS, E)
    zst = np.ascontiguousarray(zsf.T)
    vw = np.asarray(vocab_w, np.float32)
    in_maps = [{"zst": zst,
                "wvt": np.ascontiguousarray(vw[c * V_SHARD:(c + 1) * V_SHARD].T)}
               for c in range(NC)]
    res = _run("vocab", _build_vocab_kernel, in_maps)
    sig = np.stack([res.results[c]["sig"] for c in range(NC)], 0)  # (8,4096,8)
    if np.any(vb):
        # general-bias fallback (not hit for the reference inputs)
        logits = zsf @ vw.T + vb
        lse = np.log(np.exp(logits).sum(-1)).reshape(B, NSTEP)
    else:
        lse = np.log(sig.sum(axis=(0, 2))).reshape(B, NSTEP).astype(np.float32)
    # exact logit at the target index, computed on host (4096 dot products)
    yf = np.asarray(y).reshape(-1)
    logit_y = (np.einsum('re,re->r', zsf, vw[yf]) + vb[yf]).reshape(B, NSTEP)
    return (logit_y - lse).astype(np.float32)


# revision 5
# speedup vs baseline: 1.0244x; 1.0244x over previous
import numpy as np
import concourse.bass as bass
import concourse.mybir as mybir
import concourse.tile as tile
from concourse import bass_utils
import bass_rust

B, E, M, V, NSTEP = 64, 512, 64, 32000, 64
NC = 8
PROJ_ROWS = 4 * M * E          # 131072 rows of concatenated proj weights
PR_SHARD = PROJ_ROWS // NC     # 16384 rows/core
V_SHARD = V // NC              # 4000 vocab rows/core
NT = 500                       # vocab n-tile (fits one PSUM bank)
ROWS = B * NSTEP               # 4096 zs rows


def _split_multi_waits(nc, max_waits=1):
    # walrus in this container rejects >1 sem-wait on CTRL_NO instructions;
    # move extra waits onto preceding NoOps on the same engine.
    for f in nc.m.functions:
        for bb in f.blocks:
            new_insts = []
            for inst in bb.instructions:
                si = inst.sync_info
                if si is not None and si.on_wait and len(si.on_wait) > max_waits:
                    waits = list(si.on_wait)
                    head, tail = waits[:-max_waits], waits[-max_waits:]
                    for i in range(0, len(head), max_waits):
                        new_insts.append(mybir.InstNoOp(
                            name=f"{inst.name}_wsplit_{i}",
                            engine=inst.engine,
                            sync_info=bass_rust.SyncInfo(
                                on_wait=head[i:i + max_waits], on_update=[]),
                        ))
                    inst.sync_info = bass_rust.SyncInfo(
                        on_wait=tail, on_update=list(si.on_update))
                new_insts.append(inst)
            if len(new_insts) != len(bb.instructions):
                bb.instructions[:] = new_insts


def _build_proj_kernel():
    nc = bass.Bass("TRN2", target_bir_lowering=False, debug=False)
    z = nc.dram_tensor("z0t", [E, B], mybir.dt.float32, kind="ExternalInput")
    w = nc.dram_tensor("wt", [E, PR_SHARD], mybir.dt.float32, kind="ExternalInput")
    o = nc.dram_tensor("out", [B, PR_SHARD], mybir.dt.float32, kind="ExternalOutput")
    with tile.TileContext(nc) as tc:
        with tc.tile_pool(name="zp", bufs=1) as zp, \
             tc.tile_pool(name="wp", bufs=4) as wp, \
             tc.tile_pool(name="wrp", bufs=4) as wrp, \
             tc.tile_pool(name="op", bufs=3) as op, \
             tc.tile_pool(name="ps", bufs=4, space="PSUM") as pp:
            zt = zp.tile([128, 4 * B], mybir.dt.float32)
            for c in range(4):
                nc.sync.dma_start(zt[:, c * B:(c + 1) * B],
                                  z[c * 128:(c + 1) * 128, :])
            ztr = zp.tile([128, 4 * B], mybir.dt.float32r)
            nc.vector.tensor_copy(ztr[:], zt[:])
            for n in range(PR_SHARD // 512):
                wt = wp.tile([128, 4 * 512], mybir.dt.float32, tag="w")
                for c in range(4):
                    nc.sync.dma_start(
                        wt[:, c * 512:(c + 1) * 512],
                        w[c * 128:(c + 1) * 128, n * 512:(n + 1) * 512])
                wtr = wrp.tile([128, 4 * 512], mybir.dt.float32r, tag="wr")
                nc.vector.tensor_copy(wtr[:], wt[:])
                ps = pp.tile([B, 512], mybir.dt.float32)
                for c in range(4):
                    nc.tensor.matmul(ps[:], ztr[:, c * B:(c + 1) * B],
                                     wtr[:, c * 512:(c + 1) * 512],
                                     start=(c == 0), stop=(c == 3))
                ot = op.tile([B, 512], mybir.dt.float32, tag="o")
                nc.scalar.copy(ot[:], ps[:])
                nc.sync.dma_start(o[:, n * 512:(n + 1) * 512], ot[:])
    _split_multi_waits(nc)
    return nc


def _build_vocab_kernel():
    nc = bass.Bass("TRN2", target_bir_lowering=False, debug=False)
    z = nc.dram_tensor("zst", [E, ROWS], mybir.dt.float32, kind="ExternalInput")
    w = nc.dram_tensor("wvt", [E, V_SHARD], mybir.dt.float32, kind="ExternalInput")
    o = nc.dram_tensor("sig", [ROWS, V_SHARD // NT], mybir.dt.float32,
                       kind="ExternalOutput")
    NTILES = V_SHARD // NT
    with tile.TileContext(nc) as tc:
        with tc.tile_pool(name="stage", bufs=2) as sp, \
             tc.tile_pool(name="zr", bufs=1) as zrp, \
             tc.tile_pool(name="wr", bufs=1) as wrp, \
             tc.tile_pool(name="sg", bufs=3) as sgp, \
             tc.tile_pool(name="ex", bufs=2) as exp_, \
             tc.tile_pool(name="ps", bufs=4, space="PSUM") as pp:
        # round both operands to f32r once, keep resident in SBUF
            zr = [zrp.tile([128, ROWS], mybir.dt.float32r, name=f"zr{c}", tag=f"zr{c}")
                  for c in range(4)]
            wr = [wrp.tile([128, V_SHARD], mybir.dt.float32r, name=f"wr{c}", tag=f"wr{c}")
                  for c in range(4)]
            for c in range(4):
                st = sp.tile([128, ROWS], mybir.dt.float32, tag="st")
                nc.sync.dma_start(st[:], z[c * 128:(c + 1) * 128, :])
                nc.vector.tensor_copy(zr[c][:], st[:])
            for c in range(4):
                st = sp.tile([128, V_SHARD], mybir.dt.float32, tag="st")
                nc.sync.dma_start(st[:], w[c * 128:(c + 1) * 128, :])
                nc.vector.tensor_copy(wr[c][:], st[:])
            for m in range(ROWS // 128):
                sg = sgp.tile([128, NTILES], mybir.dt.float32, tag="sg")
                for n in range(NTILES):
                    ps = pp.tile([128, NT], mybir.dt.float32)
                    for c in range(4):
                        nc.tensor.matmul(
                            ps[:], zr[c][:, m * 128:(m + 1) * 128],
                            wr[c][:, n * NT:(n + 1) * NT],
                            start=(c == 0), stop=(c == 3))
                    et = exp_.tile([128, NT], mybir.dt.float32, tag="et")
                    nc.scalar.activation(et[:], ps[:],
                                         mybir.ActivationFunctionType.Exp,
                                         accum_out=sg[:, n:n + 1])
                nc.sync.dma_start(o[m * 128:(m + 1) * 128, :], sg[:])
    _split_multi_waits(nc)
    return nc


_CACHE = {}
_LAST_MAPS = {}


def _run(key, builder, in_maps):
    if key not in _CACHE:
        _CACHE[key] = builder()
    _LAST_MAPS[key] = in_maps
    return bass_utils.run_bass_kernel_spmd(
        _CACHE[key], in_maps, core_ids=list(range(NC)))


def _std_norm(x):
    s = x.std(axis=-1, keepdims=True, ddof=1)
    return x / (1e-5 + s) * 0.113


def kernel(zi, y, noise, latent, emit_k_w, emit_k_b, emit_v_w, emit_v_b,
           trans_k_w, trans_k_b, trans_v_w, trans_v_b, vocab_w, vocab_b):
    zi = np.asarray(zi); y = np.asarray(y)
    noise = np.asarray(noise, np.float32)
    latent = np.asarray(latent, np.float32)

    lat = latent[zi].reshape(B, 2, E)
    lat = _std_norm(lat) + (noise - 0.5) * np.float32(0.05)
    z0 = lat[:, 0]
    z = lat[:, 1:2].copy()

    # ---- phase 1: 4 projections on device, row-sharded over 8 cores ----
    wcat = np.concatenate([np.asarray(w_, np.float32) for w_ in
                           (emit_k_w, emit_v_w, trans_k_w, trans_v_w)], axis=0)
    z0t = np.ascontiguousarray(z0.T)
    in_maps = [{"z0t": z0t,
                "wt": np.ascontiguousarray(wcat[c * PR_SHARD:(c + 1) * PR_SHARD].T)}
               for c in range(NC)]
    res = _run("proj", _build_proj_kernel, in_maps)
    pcat = np.concatenate([res.results[c]["out"] for c in range(NC)], axis=1)
    ek, ev, tk, tv = [pcat[:, i * M * E:(i + 1) * M * E].reshape(B, M, E)
                      for i in range(4)]
    ek = ek + np.asarray(emit_k_b, np.float32).reshape(1, M, E)
    ev = ev + np.asarray(emit_v_b, np.float32).reshape(1, M, E)
    tk = tk + np.asarray(trans_k_b, np.float32).reshape(1, M, E)
    tv = tv + np.asarray(trans_v_b, np.float32).reshape(1, M, E)

    # ---- phase 2: the 64-step recurrence (tiny: 64x(64,512) state) ----
    ekT = ek.transpose(0, 2, 1); tkT = tk.transpose(0, 2, 1)
    zs = np.empty((B, NSTEP, E), np.float32)
    for t in range(NSTEP):
        zn = _std_norm(z)
        le = np.matmul(zn, ekT)
        le -= le.max(axis=-1, keepdims=True)
        ae = np.exp(le); ae /= ae.sum(axis=-1, keepdims=True)
        zs[:, t] = np.matmul(ae, ev)[:, 0]
        lt = np.matmul(zn, tkT)
        lt -= lt.max(axis=-1, keepdims=True)
        at = np.exp(lt); at /= at.sum(axis=-1, keepdims=True)
        z = np.matmul(at, tv)

    # ---- phase 3: vocab head on device, vocab-sharded over 8 cores ----
    vb = np.asarray(vocab_b, np.float32)
    zsf = zs.reshape(ROWS, E)
    zst = np.ascontiguousarray(zsf.T)
    vw = np.asarray(vocab_w, np.float32)
    in_maps = [{"zst": zst,
                "wvt": np.ascontiguousarray(vw[c * V_SHARD:(c + 1) * V_SHARD].T)}
               for c in range(NC)]
    res = _run("vocab", _build_vocab_kernel, in_maps)
    sig = np.stack([res.results[c]["sig"] for c in range(NC)], 0)  # (8,4096,8)
    if np.any(vb):
        # general-bias fallback (not hit for the reference inputs)
        logits = zsf @ vw.T + vb
        lse = np.log(np.exp(logits).sum(-1)).reshape(B, NSTEP)
    else:
        lse = np.log(sig.sum(axis=(0, 2))).reshape(B, NSTEP).astype(np.float32)
    # exact logit at the target index, computed on host (4096 dot products)
    yf = np.asarray(y).reshape(-1)
    logit_y = (np.einsum('re,re->r', zsf, vw[yf]) + vb[yf]).reshape(B, NSTEP)
    return (logit_y - lse).astype(np.float32)


# revision 6
# speedup vs baseline: 1.1442x; 1.1169x over previous
import numpy as np
import concourse.bass as bass
import concourse.mybir as mybir
import concourse.tile as tile
from concourse import bass_utils
import bass_rust

B, E, M, V, NSTEP = 64, 512, 64, 32000, 64
NC = 8
PROJ_ROWS = 4 * M * E          # 131072 rows of concatenated proj weights
PR_SHARD = PROJ_ROWS // NC     # 16384 rows/core
V_SHARD = V // NC              # 4000 vocab rows/core
NT = 500                       # vocab n-tile (fits one PSUM bank)
ROWS = B * NSTEP               # 4096 zs rows


def _split_multi_waits(nc, max_waits=1):
    # walrus in this container rejects >1 sem-wait on CTRL_NO instructions;
    # move extra waits onto preceding NoOps on the same engine.
    for f in nc.m.functions:
        for bb in f.blocks:
            new_insts = []
            for inst in bb.instructions:
                si = inst.sync_info
                if si is not None and si.on_wait and len(si.on_wait) > max_waits:
                    waits = list(si.on_wait)
                    head, tail = waits[:-max_waits], waits[-max_waits:]
                    for i in range(0, len(head), max_waits):
                        new_insts.append(mybir.InstNoOp(
                            name=f"{inst.name}_wsplit_{i}",
                            engine=inst.engine,
                            sync_info=bass_rust.SyncInfo(
                                on_wait=head[i:i + max_waits], on_update=[]),
                        ))
                    inst.sync_info = bass_rust.SyncInfo(
                        on_wait=tail, on_update=list(si.on_update))
                new_insts.append(inst)
            if len(new_insts) != len(bb.instructions):
                bb.instructions[:] = new_insts


def _build_proj_kernel():
    nc = bass.Bass("TRN2", target_bir_lowering=False, debug=False)
    z = nc.dram_tensor("z0t", [E, B], mybir.dt.float32, kind="ExternalInput")
    w = nc.dram_tensor("wt", [E, PR_SHARD], mybir.dt.float32, kind="ExternalInput")
    o = nc.dram_tensor("out", [B, PR_SHARD], mybir.dt.float32, kind="ExternalOutput")
    with tile.TileContext(nc) as tc:
        with tc.tile_pool(name="zp", bufs=1) as zp, \
             tc.tile_pool(name="wp", bufs=4) as wp, \
             tc.tile_pool(name="wrp", bufs=4) as wrp, \
             tc.tile_pool(name="op", bufs=3) as op, \
             tc.tile_pool(name="ps", bufs=4, space="PSUM") as pp:
            zt = zp.tile([128, 4 * B], mybir.dt.float32)
            for c in range(4):
                nc.sync.dma_start(zt[:, c * B:(c + 1) * B],
                                  z[c * 128:(c + 1) * 128, :])
            ztr = zp.tile([128, 4 * B], mybir.dt.float32r)
            nc.vector.tensor_copy(ztr[:], zt[:])
            for n in range(PR_SHARD // 512):
                wt = wp.tile([128, 4 * 512], mybir.dt.float32, tag="w")
                eng = nc.sync if n % 2 == 0 else nc.scalar
                eng.dma_start(
                    wt[:].rearrange("p (c n) -> p c n", c=4),
                    w[:, n * 512:(n + 1) * 512].rearrange(
                        "(c p) n -> p c n", p=128))
                wtr = wrp.tile([128, 4 * 512], mybir.dt.float32r, tag="wr")
                nc.vector.tensor_copy(wtr[:], wt[:])
                ps = pp.tile([B, 512], mybir.dt.float32)
                for c in range(4):
                    nc.tensor.matmul(ps[:], ztr[:, c * B:(c + 1) * B],
                                     wtr[:, c * 512:(c + 1) * 512],
                                     start=(c == 0), stop=(c == 3))
                ot = op.tile([B, 512], mybir.dt.float32, tag="o")
                nc.scalar.copy(ot[:], ps[:])
                nc.sync.dma_start(o[:, n * 512:(n + 1) * 512], ot[:])
    _split_multi_waits(nc)
    return nc


def _build_vocab_kernel():
    nc = bass.Bass("TRN2", target_bir_lowering=False, debug=False)
    z = nc.dram_tensor("zst", [E, ROWS], mybir.dt.float32, kind="ExternalInput")
    w = nc.dram_tensor("wvt", [E, V_SHARD], mybir.dt.float32, kind="ExternalInput")
    o = nc.dram_tensor("sig", [ROWS, V_SHARD // NT], mybir.dt.float32,
                       kind="ExternalOutput")
    NTILES = V_SHARD // NT
    with tile.TileContext(nc) as tc:
        with tc.tile_pool(name="stage", bufs=2) as sp, \
             tc.tile_pool(name="zr", bufs=1) as zrp, \
             tc.tile_pool(name="wr", bufs=1) as wrp, \
             tc.tile_pool(name="sg", bufs=3) as sgp, \
             tc.tile_pool(name="ex", bufs=2) as exp_, \
             tc.tile_pool(name="ps", bufs=4, space="PSUM") as pp:
        # round both operands to f32r once, keep resident in SBUF
            zr = [zrp.tile([128, ROWS], mybir.dt.float32r, name=f"zr{c}", tag=f"zr{c}")
                  for c in range(4)]
            wr = [wrp.tile([128, V_SHARD], mybir.dt.float32r, name=f"wr{c}", tag=f"wr{c}")
                  for c in range(4)]
            for c in range(4):
                st = sp.tile([128, ROWS], mybir.dt.float32, tag="st")
                nc.sync.dma_start(st[:], z[c * 128:(c + 1) * 128, :])
                nc.vector.tensor_copy(zr[c][:], st[:])
            for c in range(4):
                st = sp.tile([128, V_SHARD], mybir.dt.float32, tag="st")
                nc.sync.dma_start(st[:], w[c * 128:(c + 1) * 128, :])
                nc.vector.tensor_copy(wr[c][:], st[:])
            for m in range(ROWS // 128):
                sg = sgp.tile([128, NTILES], mybir.dt.float32, tag="sg")
                for n in range(NTILES):
                    ps = pp.tile([128, NT], mybir.dt.float32)
                    for c in range(4):
                        nc.tensor.matmul(
                            ps[:], zr[c][:, m * 128:(m + 1) * 128],
                            wr[c][:, n * NT:(n + 1) * NT],
                            start=(c == 0), stop=(c == 3))
                    et = exp_.tile([128, NT], mybir.dt.float32, tag="et")
                    nc.scalar.activation(et[:], ps[:],
                                         mybir.ActivationFunctionType.Exp,
                                         accum_out=sg[:, n:n + 1])
                nc.sync.dma_start(o[m * 128:(m + 1) * 128, :], sg[:])
    _split_multi_waits(nc)
    return nc


_CACHE = {}
_LAST_MAPS = {}


def _run(key, builder, in_maps):
    if key not in _CACHE:
        _CACHE[key] = builder()
    _LAST_MAPS[key] = in_maps
    return bass_utils.run_bass_kernel_spmd(
        _CACHE[key], in_maps, core_ids=list(range(NC)))


def _std_norm(x):
    s = x.std(axis=-1, keepdims=True, ddof=1)
    return x / (1e-5 + s) * 0.113


def kernel(zi, y, noise, latent, emit_k_w, emit_k_b, emit_v_w, emit_v_b,
           trans_k_w, trans_k_b, trans_v_w, trans_v_b, vocab_w, vocab_b):
    zi = np.asarray(zi); y = np.asarray(y)
    noise = np.asarray(noise, np.float32)
    latent = np.asarray(latent, np.float32)

    lat = latent[zi].reshape(B, 2, E)
    lat = _std_norm(lat) + (noise - 0.5) * np.float32(0.05)
    z0 = lat[:, 0]
    z = lat[:, 1:2].copy()

    # ---- phase 1: 4 projections on device, row-sharded over 8 cores ----
    wcat = np.concatenate([np.asarray(w_, np.float32) for w_ in
                           (emit_k_w, emit_v_w, trans_k_w, trans_v_w)], axis=0)
    z0t = np.ascontiguousarray(z0.T)
    in_maps = [{"z0t": z0t,
                "wt": np.ascontiguousarray(wcat[c * PR_SHARD:(c + 1) * PR_SHARD].T)}
               for c in range(NC)]
    res = _run("proj", _build_proj_kernel, in_maps)
    pcat = np.concatenate([res.results[c]["out"] for c in range(NC)], axis=1)
    ek, ev, tk, tv = [pcat[:, i * M * E:(i + 1) * M * E].reshape(B, M, E)
                      for i in range(4)]
    ek = ek + np.asarray(emit_k_b, np.float32).reshape(1, M, E)
    ev = ev + np.asarray(emit_v_b, np.float32).reshape(1, M, E)
    tk = tk + np.asarray(trans_k_b, np.float32).reshape(1, M, E)
    tv = tv + np.asarray(trans_v_b, np.float32).reshape(1, M, E)

    # ---- phase 2: the 64-step recurrence (tiny: 64x(64,512) state) ----
    ekT = ek.transpose(0, 2, 1); tkT = tk.transpose(0, 2, 1)
    zs = np.empty((B, NSTEP, E), np.float32)
    for t in range(NSTEP):
        zn = _std_norm(z)
        le = np.matmul(zn, ekT)
        le -= le.max(axis=-1, keepdims=True)
        ae = np.exp(le); ae /= ae.sum(axis=-1, keepdims=True)
        zs[:, t] = np.matmul(ae, ev)[:, 0]
        lt = np.matmul(zn, tkT)
        lt -= lt.max(axis=-1, keepdims=True)
        at = np.exp(lt); at /= at.sum(axis=-1, keepdims=True)
        z = np.matmul(at, tv)

    # ---- phase 3: vocab head on device, vocab-sharded over 8 cores ----
    vb = np.asarray(vocab_b, np.float32)
    zsf = zs.reshape(ROWS, E)
    zst = np.ascontiguousarray(zsf.T)
    vw = np.asarray(vocab_w, np.float32)
    in_maps = [{"zst": zst,
                "wvt": np.ascontiguousarray(vw[c * V_SHARD:(c + 1) * V_SHARD].T)}
               for c in range(NC)]
    res = _run("vocab", _build_vocab_kernel, in_maps)
    sig = np.stack([res.results[c]["sig"] for c in range(NC)], 0)  # (8,4096,8)
    if np.any(vb):
        # general-bias fallback (not hit for the reference inputs)
        logits = zsf @ vw.T + vb
        lse = np.log(np.exp(logits).sum(-1)).reshape(B, NSTEP)
    else:
        lse = np.log(sig.sum(axis=(0, 2))).reshape(B, NSTEP).astype(np.float32)
    # exact logit at the target index, computed on host (4096 dot products)
    yf = np.asarray(y).reshape(-1)
    logit_y = (np.einsum('re,re->r', zsf, vw[yf]) + vb[yf]).reshape(B, NSTEP)
    return (logit_y - lse).astype(np.float32)


# revision 7
# speedup vs baseline: 1.1550x; 1.0095x over previous
import numpy as np
import concourse.bass as bass
import concourse.mybir as mybir
import concourse.tile as tile
from concourse import bass_utils
import bass_rust

B, E, M, V, NSTEP = 64, 512, 64, 32000, 64
NC = 8
PROJ_ROWS = 4 * M * E          # 131072 rows of concatenated proj weights
PR_SHARD = PROJ_ROWS // NC     # 16384 rows/core
V_SHARD = V // NC              # 4000 vocab rows/core
NT = 500                       # vocab n-tile (fits one PSUM bank)
ROWS = B * NSTEP               # 4096 zs rows


def _split_multi_waits(nc, max_waits=1):
    # walrus in this container rejects >1 sem-wait on CTRL_NO instructions;
    # move extra waits onto preceding NoOps on the same engine.
    for f in nc.m.functions:
        for bb in f.blocks:
            new_insts = []
            for inst in bb.instructions:
                si = inst.sync_info
                if si is not None and si.on_wait and len(si.on_wait) > max_waits:
                    waits = list(si.on_wait)
                    head, tail = waits[:-max_waits], waits[-max_waits:]
                    for i in range(0, len(head), max_waits):
                        new_insts.append(mybir.InstNoOp(
                            name=f"{inst.name}_wsplit_{i}",
                            engine=inst.engine,
                            sync_info=bass_rust.SyncInfo(
                                on_wait=head[i:i + max_waits], on_update=[]),
                        ))
                    inst.sync_info = bass_rust.SyncInfo(
                        on_wait=tail, on_update=list(si.on_update))
                new_insts.append(inst)
            if len(new_insts) != len(bb.instructions):
                bb.instructions[:] = new_insts


def _build_proj_kernel():
    nc = bass.Bass("TRN2", target_bir_lowering=False, debug=False)
    z = nc.dram_tensor("z0t", [E, B], mybir.dt.float32, kind="ExternalInput")
    w = nc.dram_tensor("wt", [E, PR_SHARD], mybir.dt.float32, kind="ExternalInput")
    o = nc.dram_tensor("out", [B, PR_SHARD], mybir.dt.float32, kind="ExternalOutput")
    with tile.TileContext(nc) as tc:
        with tc.tile_pool(name="zp", bufs=1) as zp, \
             tc.tile_pool(name="wp", bufs=4) as wp, \
             tc.tile_pool(name="wrp", bufs=4) as wrp, \
             tc.tile_pool(name="op", bufs=3) as op, \
             tc.tile_pool(name="ps", bufs=4, space="PSUM") as pp:
            zt = zp.tile([128, 4 * B], mybir.dt.float32)
            for c in range(4):
                nc.sync.dma_start(zt[:, c * B:(c + 1) * B],
                                  z[c * 128:(c + 1) * 128, :])
            ztr = zp.tile([128, 4 * B], mybir.dt.float32r)
            nc.vector.tensor_copy(ztr[:], zt[:])
            for n in range(PR_SHARD // 512):
                wt = wp.tile([128, 4 * 512], mybir.dt.float32, tag="w")
                eng = nc.sync if n % 2 == 0 else nc.scalar
                eng.dma_start(
                    wt[:].rearrange("p (c n) -> p c n", c=4),
                    w[:, n * 512:(n + 1) * 512].rearrange(
                        "(c p) n -> p c n", p=128))
                wtr = wrp.tile([128, 4 * 512], mybir.dt.float32r, tag="wr")
                nc.vector.tensor_copy(wtr[:], wt[:])
                ps = pp.tile([B, 512], mybir.dt.float32)
                for c in range(4):
                    nc.tensor.matmul(ps[:], ztr[:, c * B:(c + 1) * B],
                                     wtr[:, c * 512:(c + 1) * 512],
                                     start=(c == 0), stop=(c == 3))
                ot = op.tile([B, 512], mybir.dt.float32, tag="o")
                nc.scalar.copy(ot[:], ps[:])
                nc.sync.dma_start(o[:, n * 512:(n + 1) * 512], ot[:])
    _split_multi_waits(nc)
    return nc


def _build_vocab_kernel():
    nc = bass.Bass("TRN2", target_bir_lowering=False, debug=False)
    z = nc.dram_tensor("zst", [E, ROWS], mybir.dt.float32, kind="ExternalInput")
    w = nc.dram_tensor("wvt", [E, V_SHARD], mybir.dt.float32, kind="ExternalInput")
    o = nc.dram_tensor("sig", [ROWS, V_SHARD // NT], mybir.dt.float32,
                       kind="ExternalOutput")
    NTILES = V_SHARD // NT
    with tile.TileContext(nc) as tc:
        with tc.tile_pool(name="stage", bufs=2) as sp, \
             tc.tile_pool(name="zr", bufs=1) as zrp, \
             tc.tile_pool(name="wr", bufs=1) as wrp, \
             tc.tile_pool(name="sg", bufs=3) as sgp, \
             tc.tile_pool(name="ex", bufs=2) as exp_, \
             tc.tile_pool(name="ps", bufs=4, space="PSUM") as pp:
        # round both operands to f32r once, keep resident in SBUF
            zr = [zrp.tile([128, ROWS], mybir.dt.float32r, name=f"zr{c}", tag=f"zr{c}")
                  for c in range(4)]
            wr = [wrp.tile([128, V_SHARD], mybir.dt.float32r, name=f"wr{c}", tag=f"wr{c}")
                  for c in range(4)]
            for c in range(4):
                st = sp.tile([128, ROWS], mybir.dt.float32, tag="st")
                eng = nc.sync if c % 2 == 0 else nc.scalar
                eng.dma_start(st[:], z[c * 128:(c + 1) * 128, :])
                nc.vector.tensor_copy(zr[c][:], st[:])
            for c in range(4):
                st = sp.tile([128, V_SHARD], mybir.dt.float32, tag="st")
                eng = nc.sync if c % 2 == 0 else nc.scalar
                eng.dma_start(st[:], w[c * 128:(c + 1) * 128, :])
                nc.vector.tensor_copy(wr[c][:], st[:])
            for m in range(ROWS // 128):
                sg = sgp.tile([128, NTILES], mybir.dt.float32, tag="sg")
                for n in range(NTILES):
                    ps = pp.tile([128, NT], mybir.dt.float32)
                    for c in range(4):
                        nc.tensor.matmul(
                            ps[:], zr[c][:, m * 128:(m + 1) * 128],
                            wr[c][:, n * NT:(n + 1) * NT],
                            start=(c == 0), stop=(c == 3))
                    et = exp_.tile([128, NT], mybir.dt.float32, tag="et")
                    nc.scalar.activation(et[:], ps[:],
                                         mybir.ActivationFunctionType.Exp,
                                         accum_out=sg[:, n:n + 1])
                nc.sync.dma_start(o[m * 128:(m + 1) * 128, :], sg[:])
    _split_multi_waits(nc)
    return nc


_CACHE = {}
_LAST_MAPS = {}


def _run(key, builder, in_maps):
    if key not in _CACHE:
        _CACHE[key] = builder()
    _LAST_MAPS[key] = in_maps
    return bass_utils.run_bass_kernel_spmd(
        _CACHE[key], in_maps, core_ids=list(range(NC)))


def _std_norm(x):
    s = x.std(axis=-1, keepdims=True, ddof=1)
    return x / (1e-5 + s) * 0.113


def kernel(zi, y, noise, latent, emit_k_w, emit_k_b, emit_v_w, emit_v_b,
           trans_k_w, trans_k_b, trans_v_w, trans_v_b, vocab_w, vocab_b):
    zi = np.asarray(zi); y = np.asarray(y)
    noise = np.asarray(noise, np.float32)
    latent = np.asarray(latent, np.float32)

    lat = latent[zi].reshape(B, 2, E)
    lat = _std_norm(lat) + (noise - 0.5) * np.float32(0.05)
    z0 = lat[:, 0]
    z = lat[:, 1:2].copy()

    # ---- phase 1: 4 projections on device, row-sharded over 8 cores ----
    wcat = np.concatenate([np.asarray(w_, np.float32) for w_ in
                           (emit_k_w, emit_v_w, trans_k_w, trans_v_w)], axis=0)
    z0t = np.ascontiguousarray(z0.T)
    in_maps = [{"z0t": z0t,
                "wt": np.ascontiguousarray(wcat[c * PR_SHARD:(c + 1) * PR_SHARD].T)}
               for c in range(NC)]
    res = _run("proj", _build_proj_kernel, in_maps)
    pcat = np.concatenate([res.results[c]["out"] for c in range(NC)], axis=1)
    ek, ev, tk, tv = [pcat[:, i * M * E:(i + 1) * M * E].reshape(B, M, E)
                      for i in range(4)]
    ek = ek + np.asarray(emit_k_b, np.float32).reshape(1, M, E)
    ev = ev + np.asarray(emit_v_b, np.float32).reshape(1, M, E)
    tk = tk + np.asarray(trans_k_b, np.float32).reshape(1, M, E)
    tv = tv + np.asarray(trans_v_b, np.float32).reshape(1, M, E)

    # ---- phase 2: the 64-step recurrence (tiny: 64x(64,512) state) ----
    ekT = ek.transpose(0, 2, 1); tkT = tk.transpose(0, 2, 1)
    zs = np.empty((B, NSTEP, E), np.float32)
    for t in range(NSTEP):
        zn = _std_norm(z)
        le = np.matmul(zn, ekT)
        le -= le.max(axis=-1, keepdims=True)
        ae = np.exp(le); ae /= ae.sum(axis=-1, keepdims=True)
        zs[:, t] = np.matmul(ae, ev)[:, 0]
        lt = np.matmul(zn, tkT)
        lt -= lt.max(axis=-1, keepdims=True)
        at = np.exp(lt); at /= at.sum(axis=-1, keepdims=True)
        z = np.matmul(at, tv)

    # ---- phase 3: vocab head on device, vocab-sharded over 8 cores ----
    vb = np.asarray(vocab_b, np.float32)
    zsf = zs.reshape(ROWS, E)
    zst = np.ascontiguousarray(zsf.T)
    vw = np.asarray(vocab_w, np.float32)
    in_maps = [{"zst": zst,
                "wvt": np.ascontiguousarray(vw[c * V_SHARD:(c + 1) * V_SHARD].T)}
               for c in range(NC)]
    res = _run("vocab", _build_vocab_kernel, in_maps)
    sig = np.stack([res.results[c]["sig"] for c in range(NC)], 0)  # (8,4096,8)
    if np.any(vb):
        # general-bias fallback (not hit for the reference inputs)
        logits = zsf @ vw.T + vb
        lse = np.log(np.exp(logits).sum(-1)).reshape(B, NSTEP)
    else:
        lse = np.log(sig.sum(axis=(0, 2))).reshape(B, NSTEP).astype(np.float32)
    # exact logit at the target index, computed on host (4096 dot products)
    yf = np.asarray(y).reshape(-1)
    logit_y = (np.einsum('re,re->r', zsf, vw[yf]) + vb[yf]).reshape(B, NSTEP)
    return (logit_y - lse).astype(np.float32)


# revision 8
# speedup vs baseline: 1.4739x; 1.2760x over previous
import numpy as np
import ml_dtypes
BF16 = ml_dtypes.bfloat16
import concourse.bass as bass
import concourse.mybir as mybir
import concourse.tile as tile
from concourse import bass_utils
import bass_rust

B, E, M, V, NSTEP = 64, 512, 64, 32000, 64
NC = 8
PROJ_ROWS = 4 * M * E          # 131072 rows of concatenated proj weights
PR_SHARD = PROJ_ROWS // NC     # 16384 rows/core
V_SHARD = V // NC              # 4000 vocab rows/core
NT = 500                       # vocab n-tile (fits one PSUM bank)
ROWS = B * NSTEP               # 4096 zs rows


def _split_multi_waits(nc, max_waits=1):
    # walrus in this container rejects >1 sem-wait on CTRL_NO instructions;
    # move extra waits onto preceding NoOps on the same engine.
    for f in nc.m.functions:
        for bb in f.blocks:
            new_insts = []
            for inst in bb.instructions:
                si = inst.sync_info
                if si is not None and si.on_wait and len(si.on_wait) > max_waits:
                    waits = list(si.on_wait)
                    head, tail = waits[:-max_waits], waits[-max_waits:]
                    for i in range(0, len(head), max_waits):
                        new_insts.append(mybir.InstNoOp(
                            name=f"{inst.name}_wsplit_{i}",
                            engine=inst.engine,
                            sync_info=bass_rust.SyncInfo(
                                on_wait=head[i:i + max_waits], on_update=[]),
                        ))
                    inst.sync_info = bass_rust.SyncInfo(
                        on_wait=tail, on_update=list(si.on_update))
                new_insts.append(inst)
            if len(new_insts) != len(bb.instructions):
                bb.instructions[:] = new_insts


def _build_proj_kernel():
    nc = bass.Bass("TRN2", target_bir_lowering=False, debug=False)
    z = nc.dram_tensor("z0t", [E, B], mybir.dt.bfloat16, kind="ExternalInput")
    w = nc.dram_tensor("wt", [E, PR_SHARD], mybir.dt.bfloat16, kind="ExternalInput")
    o = nc.dram_tensor("out", [B, PR_SHARD], mybir.dt.float32, kind="ExternalOutput")
    with tile.TileContext(nc) as tc:
        with tc.tile_pool(name="zp", bufs=1) as zp, \
             tc.tile_pool(name="wp", bufs=4) as wp, \
             tc.tile_pool(name="wrp", bufs=4) as wrp, \
             tc.tile_pool(name="op", bufs=3) as op, \
             tc.tile_pool(name="ps", bufs=4, space="PSUM") as pp:
            zt = zp.tile([128, 4 * B], mybir.dt.bfloat16)
            for c in range(4):
                nc.sync.dma_start(zt[:, c * B:(c + 1) * B],
                                  z[c * 128:(c + 1) * 128, :])
            for n in range(PR_SHARD // 512):
                wt = wp.tile([128, 4 * 512], mybir.dt.bfloat16, tag="w")
                eng = nc.sync if n % 2 == 0 else nc.scalar
                eng.dma_start(
                    wt[:].rearrange("p (c n) -> p c n", c=4),
                    w[:, n * 512:(n + 1) * 512].rearrange(
                        "(c p) n -> p c n", p=128))
                ps = pp.tile([B, 512], mybir.dt.float32)
                for c in range(4):
                    nc.tensor.matmul(ps[:], zt[:, c * B:(c + 1) * B],
                                     wt[:, c * 512:(c + 1) * 512],
                                     start=(c == 0), stop=(c == 3))
                ot = op.tile([B, 512], mybir.dt.float32, tag="o")
                nc.scalar.copy(ot[:], ps[:])
                nc.sync.dma_start(o[:, n * 512:(n + 1) * 512], ot[:])
    _split_multi_waits(nc)
    return nc


def _build_vocab_kernel():
    nc = bass.Bass("TRN2", target_bir_lowering=False, debug=False)
    z = nc.dram_tensor("zst", [E, ROWS], mybir.dt.bfloat16, kind="ExternalInput")
    w = nc.dram_tensor("wvt", [E, V_SHARD], mybir.dt.bfloat16, kind="ExternalInput")
    o = nc.dram_tensor("sig", [ROWS, V_SHARD // NT], mybir.dt.float32,
                       kind="ExternalOutput")
    NTILES = V_SHARD // NT
    with tile.TileContext(nc) as tc:
        with tc.tile_pool(name="stage", bufs=2) as sp, \
             tc.tile_pool(name="zr", bufs=1) as zrp, \
             tc.tile_pool(name="wr", bufs=1) as wrp, \
             tc.tile_pool(name="sg", bufs=3) as sgp, \
             tc.tile_pool(name="ex", bufs=2) as exp_, \
             tc.tile_pool(name="ps", bufs=4, space="PSUM") as pp:
        # round both operands to f32r once, keep resident in SBUF
            zr = [zrp.tile([128, ROWS], mybir.dt.bfloat16, name=f"zr{c}", tag=f"zr{c}")
                  for c in range(4)]
            wr = [wrp.tile([128, V_SHARD], mybir.dt.bfloat16, name=f"wr{c}", tag=f"wr{c}")
                  for c in range(4)]
            for c in range(4):
                eng = nc.sync if c % 2 == 0 else nc.scalar
                eng.dma_start(zr[c][:], z[c * 128:(c + 1) * 128, :])
            for c in range(4):
                eng = nc.sync if c % 2 == 0 else nc.scalar
                eng.dma_start(wr[c][:], w[c * 128:(c + 1) * 128, :])
            for m in range(ROWS // 128):
                sg = sgp.tile([128, NTILES], mybir.dt.float32, tag="sg")
                for n in range(NTILES):
                    ps = pp.tile([128, NT], mybir.dt.float32)
                    for c in range(4):
                        nc.tensor.matmul(
                            ps[:], zr[c][:, m * 128:(m + 1) * 128],
                            wr[c][:, n * NT:(n + 1) * NT],
                            start=(c == 0), stop=(c == 3))
                    et = exp_.tile([128, NT], mybir.dt.float32, tag="et")
                    nc.scalar.activation(et[:], ps[:],
                                         mybir.ActivationFunctionType.Exp,
                                         accum_out=sg[:, n:n + 1])
                nc.sync.dma_start(o[m * 128:(m + 1) * 128, :], sg[:])
    _split_multi_waits(nc)
    return nc


_CACHE = {}
_LAST_MAPS = {}


def _run(key, builder, in_maps):
    if key not in _CACHE:
        _CACHE[key] = builder()
    _LAST_MAPS[key] = in_maps
    return bass_utils.run_bass_kernel_spmd(
        _CACHE[key], in_maps, core_ids=list(range(NC)))


def _std_norm(x):
    s = x.std(axis=-1, keepdims=True, ddof=1)
    return x / (1e-5 + s) * 0.113


def kernel(zi, y, noise, latent, emit_k_w, emit_k_b, emit_v_w, emit_v_b,
           trans_k_w, trans_k_b, trans_v_w, trans_v_b, vocab_w, vocab_b):
    zi = np.asarray(zi); y = np.asarray(y)
    noise = np.asarray(noise, np.float32)
    latent = np.asarray(latent, np.float32)

    lat = latent[zi].reshape(B, 2, E)
    lat = _std_norm(lat) + (noise - 0.5) * np.float32(0.05)
    z0 = lat[:, 0]
    z = lat[:, 1:2].copy()

    # ---- phase 1: 4 projections on device, row-sharded over 8 cores ----
    wcat = np.concatenate([np.asarray(w_, np.float32) for w_ in
                           (emit_k_w, emit_v_w, trans_k_w, trans_v_w)], axis=0)
    z0t = np.ascontiguousarray(z0.T).astype(BF16)
    in_maps = [{"z0t": z0t,
                "wt": np.ascontiguousarray(wcat[c * PR_SHARD:(c + 1) * PR_SHARD].T).astype(BF16)}
               for c in range(NC)]
    res = _run("proj", _build_proj_kernel, in_maps)
    pcat = np.concatenate([res.results[c]["out"] for c in range(NC)], axis=1)
    ek, ev, tk, tv = [pcat[:, i * M * E:(i + 1) * M * E].reshape(B, M, E)
                      for i in range(4)]
    ek = ek + np.asarray(emit_k_b, np.float32).reshape(1, M, E)
    ev = ev + np.asarray(emit_v_b, np.float32).reshape(1, M, E)
    tk = tk + np.asarray(trans_k_b, np.float32).reshape(1, M, E)
    tv = tv + np.asarray(trans_v_b, np.float32).reshape(1, M, E)

    # ---- phase 2: the 64-step recurrence (tiny: 64x(64,512) state) ----
    ekT = ek.transpose(0, 2, 1); tkT = tk.transpose(0, 2, 1)
    zs = np.empty((B, NSTEP, E), np.float32)
    for t in range(NSTEP):
        zn = _std_norm(z)
        le = np.matmul(zn, ekT)
        le -= le.max(axis=-1, keepdims=True)
        ae = np.exp(le); ae /= ae.sum(axis=-1, keepdims=True)
        zs[:, t] = np.matmul(ae, ev)[:, 0]
        lt = np.matmul(zn, tkT)
        lt -= lt.max(axis=-1, keepdims=True)
        at = np.exp(lt); at /= at.sum(axis=-1, keepdims=True)
        z = np.matmul(at, tv)

    # ---- phase 3: vocab head on device, vocab-sharded over 8 cores ----
    vb = np.asarray(vocab_b, np.float32)
    zsf = zs.reshape(ROWS, E)
    zst = np.ascontiguousarray(zsf.T).astype(BF16)
    vw = np.asarray(vocab_w, np.float32)
    in_maps = [{"zst": zst,
                "wvt": np.ascontiguousarray(vw[c * V_SHARD:(c + 1) * V_SHARD].T).astype(BF16)}
               for c in range(NC)]
    res = _run("vocab", _build_vocab_kernel, in_maps)
    sig = np.stack([res.results[c]["sig"] for c in range(NC)], 0)  # (8,4096,8)
    if np.any(vb):
        # general-bias fallback (not hit for the reference inputs)
        logits = zsf @ vw.T + vb
        lse = np.log(np.exp(logits).sum(-1)).reshape(B, NSTEP)
    else:
        lse = np.log(sig.sum(axis=(0, 2))).reshape(B, NSTEP).astype(np.float32)
    # exact logit at the target index, computed on host (4096 dot products)
    yf = np.asarray(y).reshape(-1)
    logit_y = (np.einsum('re,re->r', zsf, vw[yf]) + vb[yf]).reshape(B, NSTEP)
    return (logit_y - lse).astype(np.float32)
